# revision 1
# baseline (speedup 1.0000x reference)
"""Trainium2 Bass kernel for CompressedLinear (VQ codebook linear layer).

Computes: out = x @ W^T + bias, where
  W = (centroids[indices] @ Pi) * row_norms[:, None]

Sharding: out_features (4096) split across 8 cores (512 each); x replicated.
Per-core device pipeline:
  1. Gather yts[j,o] = centroids[idxT[j,o]] via fused custom-DVE ops (2
     codebook entries per instruction, 8 instructions per tile).
  2. W_u^T[i,o] = sum_j Pi[j,i] * yts[j,o] on the PE (bf16, f32 psum).
  3. outT[o,t] = sum_i W_u^T[i,o] * xT[i,t]; then out = rn*acc + bias on DVE.
Host feeds x pre-transposed/bf16-cast (layout prep), Pi in column-stripe
layout, indices transposed; host reassembles the 8 outT shards.
"""

import numpy as np

# Problem geometry (hardcoded per contract)
OUT, IN = 4096, 4096
B, S = 4, 2048
T = B * S          # 8192 tokens
NCORES = 8
P = 128            # partitions

_DVE_OPS = None
_NC_CACHE = {}


def _register_dve_ops():
    """Register the fused VQ-gather ops in dve_ops.OPS (idempotent).

    5 ops per [P, osh] tile: VQ_Q0 covers codebook entries 0-3 (values in
    C0/C1/C2 + a 4th spilled to Src1 per the C3 stopgap), then
    VQ_ACC3_{4,7,10,13} each accumulate three more entries with hardcoded
    comparand bases (stream-invariant One-chains, hoisted to latches).
    """
    global _DVE_OPS
    if _DVE_OPS is not None:
        return _DVE_OPS
    import concourse.dve_ops as dvo
    from concourse.dve_spec import (
        Spec, Src0, Src1, C0, C1, C2, Zero, One, eq, lower,
        _spill_c3_to_src1, C3,
    )
    from concourse.dve_uop import DveOpSpec

    existing = {op.name: op for op in dvo.OPS}
    if "VQ_PAIR" in existing:
        _DVE_OPS = {k: existing[k] for k in ("VQ_PAIR", "VQ_ACC2")}
        return _DVE_OPS

    ver = "v3"  # TRN2

    def mk(name, spec, rd1):
        opcode = dvo._CUSTOM_DVE_ROW_BASE + len(dvo.OPS)
        dvo._SUB_OPCODE_FOR_NAME[name] = opcode
        s = DveOpSpec(name=name, opcode=opcode, uops=lower(spec, ver=ver), rd1_en=rd1)
        op = dvo.DveOp(name, spec, subdim=False, uops_sha={ver: s.sha(ver)})
        dvo.OPS.append(op)
        dvo.CUSTOM_DVE_SPECS[name] = spec
        return op

    # out = (idx==imm2)*s0 + (idx==imm2+1)*s1
    pair = mk(
        "VQ_PAIR",
        Spec(
            body=eq(Src0, C2) * C0 + eq(Src0, C2 + One) * C1,
            reference=lambda in0, in1, s0, s1, imm2: (
                (in0 == imm2) * s0 + (in0 == imm2 + 1) * s1
            ).astype(np.float32),
        ),
        False,
    )
    # out = acc + (idx==imm2)*s0 + (idx==imm2+1)*s1
    acc = mk(
        "VQ_ACC2",
        Spec(
            body=Src1 + eq(Src0, C2) * C0 + eq(Src0, C2 + One) * C1,
            reference=lambda in0, in1, s0, s1, imm2: (
                in1 + (in0 == imm2) * s0 + (in0 == imm2 + 1) * s1
            ).astype(np.float32),
        ),
        True,
    )
    _DVE_OPS = {"VQ_PAIR": pair, "VQ_ACC2": acc}
    return _DVE_OPS


def build_nc(cvals, in_=IN, t=T, osh=OUT // NCORES, tch=512, tg=1024):
    """Build the SPMD Bass program. cvals: 16 python floats (codebook)."""
    import concourse.bacc as bacc
    import concourse.mybir as mybir
    from concourse.tile import TileContext

    f32 = mybir.dt.float32
    bf16 = mybir.dt.bfloat16

    nj = in_ // P          # j blocks (rows of Pi / x input dim)
    ni = in_ // P          # i blocks (cols of Pi / contraction of main mm)
    nob = osh // P         # output feature blocks per core
    npass = 4              # stage-2 passes (8 psum banks each)
    nk = ni // npass       # i-blocks per stage-2 pass
    ngt = t // tg          # stage-3 token groups
    nh = tg // tch         # psum-width chunks per token group

    nc = bacc.Bacc()
    xT_d = nc.dram_tensor("xT", [in_, t], bf16, kind="ExternalInput")
    piR_d = nc.dram_tensor("PiR", [npass * nj, P, nk, P], bf16,
                           kind="ExternalInput")
    idxT_d = nc.dram_tensor("idxT", [in_, osh], bf16, kind="ExternalInput")
    rn_d = nc.dram_tensor("rn", [osh], f32, kind="ExternalInput")
    bias_d = nc.dram_tensor("bias", [osh], f32, kind="ExternalInput")
    outT_d = nc.dram_tensor("outT", [osh, t], f32, kind="ExternalOutput")

    with TileContext(nc) as tc:
        with (
            tc.tile_pool(name="constp", bufs=1) as constp,
            tc.tile_pool(name="idxp", bufs=3) as idxp,
            tc.tile_pool(name="ytsp", bufs=1) as ytsp,
            tc.tile_pool(name="pip", bufs=4) as pip,
            tc.tile_pool(name="wtp", bufs=1) as wtp,
            tc.tile_pool(name="xrp", bufs=6) as xrp,
            tc.tile_pool(name="outp", bufs=4) as outp,
            tc.tile_pool(name="wpsum", bufs=1, space="PSUM") as wpsum,
        ):
            rn_sb = constp.tile([P, nob], f32, name="rn_sb")
            nc.sync.dma_start(rn_sb[:], rn_d.rearrange("(b p) -> p b", p=P))
            bias_sb = constp.tile([P, nob], f32, name="bias_sb")
            nc.sync.dma_start(bias_sb[:], bias_d.rearrange("(b p) -> p b", p=P))

            # ---- Stage 1: codebook gather: yts[j][p, o] = centroids[idxT] --
            # Fused custom-DVE ops: VQ_PAIR covers entries {0,1}; each
            # VQ_ACC2 accumulates two more. 8 instructions per tile, all
            # bf16 (exact: per element exactly one eq-term is nonzero, so
            # every partial sum is 0 or bf16(c_k) — no rounding drift).
            ops = _register_dve_ops()
            vq_pair, vq_acc2 = ops["VQ_PAIR"], ops["VQ_ACC2"]
            yts = []
            for j in range(nj):
                idx_t = idxp.tile([P, osh], bf16, name="idx_t", tag="idx",
                                  bufs=3)
                nc.sync.dma_start(idx_t[:], idxT_d[j * P:(j + 1) * P, :])
                cur = idxp.tile([P, osh], bf16, name="g", tag="g", bufs=2)
                nc.vector._custom_dve(
                    vq_pair, out=cur[:], in0=idx_t[:],
                    s0=float(cvals[0]), s1=float(cvals[1]), imm2=0.0,
                )
                for k in range(2, 16, 2):
                    if k == 14:
                        dst = ytsp.tile([P, osh], bf16, name="y_t",
                                        tag=f"yts{j}")
                    else:
                        dst = idxp.tile([P, osh], bf16, name="g", tag="g",
                                        bufs=2)
                    nc.vector._custom_dve(
                        vq_acc2, out=dst[:], in0=idx_t[:], in1=cur[:],
                        s0=float(cvals[k]), s1=float(cvals[k + 1]),
                        imm2=float(k),
                    )
                    cur = dst
                yts.append(cur)

            # ---- Stage 2: wt[i_blk][p_i, o] = sum_j Pi[j, i] * yts[j, o] ---
            # j-major over 8 psum banks: pass p covers i-blocks 8p..8p+7,
            # consuming yts[j] as the gather produces them (pass 0 absorbs
            # PE work into the gather window). Pi arrives as per-(pass, j)
            # interleaved [P, 8, P] tiles (1MB resident vs 6MB stripes).
            wts = [None] * ni
            for p4 in range(npass):
                ps = [
                    wpsum.tile([P, osh], f32, name="wps", tag=f"wps{k}")
                    for k in range(nk)
                ]
                for j in range(nj):
                    pi_t = pip.tile([P, nk, P], bf16, name="pi_t", tag="pi")
                    nc.sync.dma_start(pi_t[:], piR_d[p4 * nj + j])
                    for k in range(nk):
                        nc.tensor.matmul(
                            ps[k][:], pi_t[:, k, :], yts[j][:],
                            start=(j == 0), stop=(j == nj - 1),
                        )
                for k in range(nk):
                    i_blk = p4 * nk + k
                    wt_t = wtp.tile([P, osh], bf16, name="wt_t",
                                    tag=f"wt{i_blk}")
                    nc.scalar.copy(wt_t[:], ps[k][:])
                    wts[i_blk] = wt_t

            # ---- Stage 3: outT[o, t] = rn[o] * sum_i wt[i,o]*xT[i,t] + b[o]
            # i-outer over a [tg]-token group with nob*nh=8 open psum
            # chains; per (i, ob) the stationary loads once and issues nh
            # matmuls (ldweights elided on the trailing ones). x streams as
            # [P, tg] row tiles.
            for g in range(ngt):
                mps = [
                    wpsum.tile([P, tch], f32, name="mp", tag=f"wps{c}")
                    for c in range(nob * nh)
                ]
                for i_blk in range(ni):
                    xr = xrp.tile([P, tg], bf16, name="xr", tag="xr")
                    nc.sync.dma_start(
                        xr[:],
                        xT_d[i_blk * P:(i_blk + 1) * P, g * tg:(g + 1) * tg],
                    )
                    for ob in range(nob):
                        st = wts[i_blk][:, ob * P:(ob + 1) * P]
                        for h in range(nh):
                            mi = nc.tensor.matmul(
                                mps[ob * nh + h][:], st,
                                xr[:, h * tch:(h + 1) * tch],
                                start=(i_blk == 0), stop=(i_blk == ni - 1),
                            )
                            if h > 0:
                                mi.ins.ldweights = False
                for ob in range(nob):
                    for h in range(nh):
                        o_t = outp.tile([P, tch], f32, name="o_t", tag="out")
                        nc.vector.tensor_scalar(
                            o_t[:], mps[ob * nh + h][:], rn_sb[:, ob:ob + 1],
                            bias_sb[:, ob:ob + 1],
                            mybir.AluOpType.mult, mybir.AluOpType.add,
                        )
                        nc.scalar.dma_start(
                            outT_d[ob * P:(ob + 1) * P,
                                   g * tg + h * tch:g * tg + (h + 1) * tch],
                            o_t[:],
                        )
    nc.compile()
    return nc


def _prep_inputs(x, indices, Pi, row_norms, bias):
    """Host-side layout prep + sharding. Returns list of per-core in_maps."""
    import ml_dtypes

    bf16 = ml_dtypes.bfloat16
    x2 = np.ascontiguousarray(
        np.asarray(x, np.float32).reshape(T, IN).T
    ).astype(bf16)  # (IN, T)
    nj = IN // P
    npass, nk = 4, (IN // P) // 4
    # piR[p*nj + j, jp, k, ii] = Pi[j*P + jp, (p*nk + k)*P + ii]
    piR = np.ascontiguousarray(
        np.asarray(Pi, np.float32).astype(bf16)
        .reshape(nj, P, npass, nk, P).transpose(2, 0, 1, 3, 4)
        .reshape(npass * nj, P, nk, P)
    )
    idxT = np.ascontiguousarray(np.asarray(indices).T).astype(bf16)  # (IN, OUT)
    rn = np.asarray(row_norms, np.float32)
    bs = np.asarray(bias, np.float32)

    osh = OUT // NCORES
    in_maps = []
    for c in range(NCORES):
        sl = slice(c * osh, (c + 1) * osh)
        in_maps.append({
            "xT": x2,
            "PiR": piR,
            "idxT": np.ascontiguousarray(idxT[:, sl]),
            "rn": np.ascontiguousarray(rn[sl]),
            "bias": np.ascontiguousarray(bs[sl]),
        })
    return in_maps


def _get_nc(centroids):
    key = np.asarray(centroids, np.float32).tobytes()
    nc = _NC_CACHE.get(key)
    if nc is None:
        cvals = [float(v) for v in np.asarray(centroids, np.float32)]
        assert len(cvals) == 16
        nc = build_nc(cvals)
        _NC_CACHE.clear()
        _NC_CACHE[key] = nc
    return nc


def kernel(x, indices, centroids, Pi, row_norms, bias):
    from concourse.bass_utils import run_bass_kernel_spmd

    nc = _get_nc(centroids)
    in_maps = _prep_inputs(x, indices, Pi, row_norms, bias)
    res = run_bass_kernel_spmd(nc, in_maps, list(range(NCORES)))
    shards = [np.asarray(res.results[c]["outT"]) for c in range(NCORES)]
    full = np.concatenate(shards, axis=0)           # (OUT, T)
    out = np.ascontiguousarray(full.T).reshape(B, S, OUT)
    return out.astype(np.float32)



# revision 3
# speedup vs baseline: 1.0359x; 1.0359x over previous
"""Trainium2 Bass kernel for CompressedLinear (VQ codebook linear layer).

Computes: out = x @ W^T + bias, where
  W = (centroids[indices] @ Pi) * row_norms[:, None]

Sharding: out_features (4096) split across 8 cores (512 each); x replicated.
Per-core device pipeline:
  1. Gather yts[j,o] = centroids[idxT[j,o]] via fused custom-DVE ops
     (2 codebook entries per instruction, 8 instructions per chain). Chains
     run on [128, 4096]-wide tiles (8 j-blocks packed along the free dim)
     to amortize per-op overhead; the DVE chain is the kernel's critical
     path for the first ~150us.
  2. W_u^T[i,o] = sum_j Pi[j,i] * yts[j,o] on the PE (bf16, f32 psum),
     j-major over 8 psum banks x 4 passes; pass 0 overlaps the gather.
  3. outT[o,t] = sum_i W_u^T[i,o] * xT[i,t] over 16 token groups of 512,
     4 psum chains per group alternating bank sets between groups (no
     bank-turnaround stall); out = rn*acc + bias split across DVE+GpSimd.
Host feeds x pre-transposed/bf16-cast (layout prep), Pi in column-stripe
layout, indices transposed+packed; host reassembles the 8 outT shards.
"""

import numpy as np

# Problem geometry (hardcoded per contract)
OUT, IN = 4096, 4096
B, S = 4, 2048
T = B * S          # 8192 tokens
NCORES = 8
P = 128            # partitions

_DVE_OPS = None
_NC_CACHE = {}


def _register_dve_ops():
    """Register the fused VQ-gather ops in dve_ops.OPS (idempotent).

    VQ_PAIR covers codebook entries {imm2, imm2+1}; VQ_ACC2 accumulates two
    more on top of Src1. 8 instructions cover all 16 entries. All bf16
    (exact: per element exactly one eq-term is nonzero, so every partial
    sum is 0 or bf16(c_k) — no rounding drift).
    """
    global _DVE_OPS
    if _DVE_OPS is not None:
        return _DVE_OPS
    import concourse.dve_ops as dvo
    from concourse.dve_spec import (
        Spec, Src0, Src1, C0, C1, C2, One, eq, lower,
    )
    from concourse.dve_uop import DveOpSpec

    existing = {op.name: op for op in dvo.OPS}
    if "VQ_PAIR" in existing:
        _DVE_OPS = {k: existing[k] for k in ("VQ_PAIR", "VQ_ACC2")}
        return _DVE_OPS

    ver = "v3"  # TRN2

    def mk(name, spec, rd1):
        opcode = dvo._CUSTOM_DVE_ROW_BASE + len(dvo.OPS)
        dvo._SUB_OPCODE_FOR_NAME[name] = opcode
        s = DveOpSpec(name=name, opcode=opcode, uops=lower(spec, ver=ver), rd1_en=rd1)
        op = dvo.DveOp(name, spec, subdim=False, uops_sha={ver: s.sha(ver)})
        dvo.OPS.append(op)
        dvo.CUSTOM_DVE_SPECS[name] = spec
        return op

    # out = (idx==imm2)*s0 + (idx==imm2+1)*s1
    pair = mk(
        "VQ_PAIR",
        Spec(
            body=eq(Src0, C2) * C0 + eq(Src0, C2 + One) * C1,
            reference=lambda in0, in1, s0, s1, imm2: (
                (in0 == imm2) * s0 + (in0 == imm2 + 1) * s1
            ).astype(np.float32),
        ),
        False,
    )
    # out = acc + (idx==imm2)*s0 + (idx==imm2+1)*s1
    acc = mk(
        "VQ_ACC2",
        Spec(
            body=Src1 + eq(Src0, C2) * C0 + eq(Src0, C2 + One) * C1,
            reference=lambda in0, in1, s0, s1, imm2: (
                in1 + (in0 == imm2) * s0 + (in0 == imm2 + 1) * s1
            ).astype(np.float32),
        ),
        True,
    )
    _DVE_OPS = {"VQ_PAIR": pair, "VQ_ACC2": acc}
    return _DVE_OPS


# Gather packing: JW j-blocks per wide chain, NW chains.
JW = 8
WF = JW * (OUT // NCORES)   # 4096 wide free dim
NW = (IN // P) // JW        # 4 chains


def build_nc(cvals, in_=IN, t=T, osh=OUT // NCORES, tch=512):
    """Build the SPMD Bass program. cvals: 16 python floats (codebook)."""
    import concourse.bacc as bacc
    import concourse.mybir as mybir
    from concourse.tile import TileContext

    f32 = mybir.dt.float32
    bf16 = mybir.dt.bfloat16

    nj = in_ // P          # j blocks (rows of Pi / x input dim)
    ni = in_ // P          # i blocks (cols of Pi / contraction of main mm)
    nob = osh // P         # output feature blocks per core (4)
    npass = 4              # stage-2 passes (8 psum banks each)
    nk = ni // npass       # i-blocks per stage-2 pass (8)
    ngt = t // tch         # stage-3 token groups (16)

    nc = bacc.Bacc()
    xT_d = nc.dram_tensor("xT", [in_, t], bf16, kind="ExternalInput")
    piR_d = nc.dram_tensor("PiR", [npass * nj, P, nk, P], bf16,
                           kind="ExternalInput")
    idxW_d = nc.dram_tensor("idxW", [NW, P, WF], bf16, kind="ExternalInput")
    rn_d = nc.dram_tensor("rn", [osh], f32, kind="ExternalInput")
    bias_d = nc.dram_tensor("bias", [osh], f32, kind="ExternalInput")
    outT_d = nc.dram_tensor("outT", [osh, t], f32, kind="ExternalOutput")

    with TileContext(nc) as tc:
        with (
            tc.tile_pool(name="constp", bufs=1) as constp,
            tc.tile_pool(name="idxp", bufs=2) as idxp,
            tc.tile_pool(name="ytsp", bufs=1) as ytsp,
            tc.tile_pool(name="pip", bufs=4) as pip,
            tc.tile_pool(name="wtp", bufs=1) as wtp,
            tc.tile_pool(name="xrp", bufs=6) as xrp,
            tc.tile_pool(name="outp", bufs=4) as outp,
            tc.tile_pool(name="wpsum", bufs=1, space="PSUM") as wpsum,
        ):
            # ---- Stage 1: codebook gather: yts[j][p, o] = centroids[idxT] --
            # 4 wide chains of 8 fused custom-DVE ops on [P, 4096] tiles
            # (8 j-blocks packed along the free dim). Issue the first idx
            # DMA before anything else: the DVE chain is the critical path.
            ops = _register_dve_ops()
            vq_pair, vq_acc2 = ops["VQ_PAIR"], ops["VQ_ACC2"]
            ytsw = []
            for m in range(NW):
                idx_t = idxp.tile([P, WF], bf16, name="idx_t", tag="idx",
                                  bufs=2)
                nc.sync.dma_start(idx_t[:], idxW_d[m])
                cur = idxp.tile([P, WF], bf16, name="g", tag="g", bufs=2)
                nc.vector._custom_dve(
                    vq_pair, out=cur[:], in0=idx_t[:],
                    s0=float(cvals[0]), s1=float(cvals[1]), imm2=0.0,
                )
                for k in range(2, 16, 2):
                    if k == 14:
                        dst = ytsp.tile([P, WF], bf16, name="y_t",
                                        tag=f"yts{m}")
                    else:
                        dst = idxp.tile([P, WF], bf16, name="g", tag="g",
                                        bufs=2)
                    nc.vector._custom_dve(
                        vq_acc2, out=dst[:], in0=idx_t[:], in1=cur[:],
                        s0=float(cvals[k]), s1=float(cvals[k + 1]),
                        imm2=float(k),
                    )
                    cur = dst
                ytsw.append(cur)

            rn_sb = constp.tile([P, nob], f32, name="rn_sb")
            nc.scalar.dma_start(rn_sb[:], rn_d.rearrange("(b p) -> p b", p=P))
            bias_sb = constp.tile([P, nob], f32, name="bias_sb")
            nc.scalar.dma_start(bias_sb[:],
                                bias_d.rearrange("(b p) -> p b", p=P))

            def yts_view(j):
                return ytsw[j // JW][:, (j % JW) * osh:(j % JW + 1) * osh]

            # ---- Stage 2: wt[i_blk][p_i, o] = sum_j Pi[j, i] * yts[j, o] ---
            # j-major over 8 psum banks: pass p covers i-blocks 8p..8p+7,
            # consuming yts as the gather produces them (pass 0 absorbs
            # PE work into the gather window). Pi arrives as per-(pass, j)
            # interleaved [P, 8, P] tiles (1MB resident vs 6MB stripes).
            wts = [None] * ni
            for p4 in range(npass):
                ps = [
                    wpsum.tile([P, osh], f32, name="wps", tag=f"wps{k}")
                    for k in range(nk)
                ]
                for j in range(nj):
                    pi_t = pip.tile([P, nk, P], bf16, name="pi_t", tag="pi")
                    nc.sync.dma_start(pi_t[:], piR_d[p4 * nj + j])
                    yv = yts_view(j)
                    for k in range(nk):
                        nc.tensor.matmul(
                            ps[k][:], pi_t[:, k, :], yv,
                            start=(j == 0), stop=(j == nj - 1),
                        )
                for k in range(nk):
                    i_blk = p4 * nk + k
                    wt_t = wtp.tile([P, osh], bf16, name="wt_t",
                                    tag=f"wt{i_blk}")
                    nc.scalar.copy(wt_t[:], ps[k][:])
                    wts[i_blk] = wt_t

            # ---- Stage 3: outT[o, t] = rn[o] * sum_i wt[i,o]*xT[i,t] + b[o]
            # 16 groups of 512 tokens; per group 4 psum chains (one per
            # 128-row output block), alternating bank sets between
            # consecutive groups so group g+1's matmuls never wait on
            # group g's drain. Drain work (scale+bias) alternates between
            # DVE and GpSimd; xr DMAs alternate sync/scalar queues.
            for g in range(ngt):
                bank = (g % 2) * nob
                mps = [
                    wpsum.tile([P, tch], f32, name="mp",
                               tag=f"wps{bank + ob}")
                    for ob in range(nob)
                ]
                for i_blk in range(ni):
                    xr = xrp.tile([P, tch], bf16, name="xr", tag="xr")
                    eng = nc.sync if (i_blk % 2 == 0) else nc.gpsimd
                    eng.dma_start(
                        xr[:],
                        xT_d[i_blk * P:(i_blk + 1) * P,
                             g * tch:(g + 1) * tch],
                    )
                    for ob in range(nob):
                        nc.tensor.matmul(
                            mps[ob][:], wts[i_blk][:, ob * P:(ob + 1) * P],
                            xr[:],
                            start=(i_blk == 0), stop=(i_blk == ni - 1),
                        )
                for ob in range(nob):
                    o_t = outp.tile([P, tch], f32, name="o_t", tag="out")
                    if ob % 2 == 0:
                        nc.vector.tensor_scalar(
                            o_t[:], mps[ob][:], rn_sb[:, ob:ob + 1],
                            bias_sb[:, ob:ob + 1],
                            mybir.AluOpType.mult, mybir.AluOpType.add,
                        )
                    else:
                        # out = Identity(in*scale + bias) on the Act engine
                        nc.scalar.activation(
                            o_t[:], mps[ob][:],
                            mybir.ActivationFunctionType.Identity,
                            bias=bias_sb[:, ob:ob + 1],
                            scale=rn_sb[:, ob:ob + 1],
                        )
                    nc.scalar.dma_start(
                        outT_d[ob * P:(ob + 1) * P,
                               g * tch:(g + 1) * tch],
                        o_t[:],
                    )
    nc.compile()
    return nc


def _prep_inputs(x, indices, Pi, row_norms, bias):
    """Host-side layout prep + sharding. Returns list of per-core in_maps."""
    import ml_dtypes

    bf16 = ml_dtypes.bfloat16
    x2 = np.ascontiguousarray(
        np.asarray(x, np.float32).reshape(T, IN).T
    ).astype(bf16)  # (IN, T)
    nj = IN // P
    npass, nk = 4, (IN // P) // 4
    # piR[p*nj + j, jp, k, ii] = Pi[j*P + jp, (p*nk + k)*P + ii]
    piR = np.ascontiguousarray(
        np.asarray(Pi, np.float32).astype(bf16)
        .reshape(nj, P, npass, nk, P).transpose(2, 0, 1, 3, 4)
        .reshape(npass * nj, P, nk, P)
    )
    idxT = np.ascontiguousarray(np.asarray(indices).T).astype(bf16)  # (IN, OUT)
    rn = np.asarray(row_norms, np.float32)
    bs = np.asarray(bias, np.float32)

    osh = OUT // NCORES
    in_maps = []
    for c in range(NCORES):
        sl = slice(c * osh, (c + 1) * osh)
        # idxW[m, p, q*osh + o] = idxT[(m*JW + q)*P + p, c*osh + o]
        idxW = np.ascontiguousarray(
            idxT[:, sl].reshape(NW, JW, P, osh).transpose(0, 2, 1, 3)
            .reshape(NW, P, WF)
        )
        in_maps.append({
            "xT": x2,
            "PiR": piR,
            "idxW": idxW,
            "rn": np.ascontiguousarray(rn[sl]),
            "bias": np.ascontiguousarray(bs[sl]),
        })
    return in_maps


def _get_nc(centroids):
    key = np.asarray(centroids, np.float32).tobytes()
    nc = _NC_CACHE.get(key)
    if nc is None:
        cvals = [float(v) for v in np.asarray(centroids, np.float32)]
        assert len(cvals) == 16
        nc = build_nc(cvals)
        _NC_CACHE.clear()
        _NC_CACHE[key] = nc
    return nc


def kernel(x, indices, centroids, Pi, row_norms, bias):
    from concourse.bass_utils import run_bass_kernel_spmd

    nc = _get_nc(centroids)
    in_maps = _prep_inputs(x, indices, Pi, row_norms, bias)
    res = run_bass_kernel_spmd(nc, in_maps, list(range(NCORES)))
    shards = [np.asarray(res.results[c]["outT"]) for c in range(NCORES)]
    full = np.concatenate(shards, axis=0)           # (OUT, T)
    out = np.ascontiguousarray(full.T).reshape(B, S, OUT)
    return out.astype(np.float32)


# revision 10
# speedup vs baseline: 1.0654x; 1.0284x over previous
"""Trainium2 Bass kernel for CompressedLinear (VQ codebook linear layer).

Computes: out = x @ W^T + bias, where
  W = (centroids[indices] @ Pi) * row_norms[:, None]

Sharding: out_features (4096) split across 8 cores (512 each); x replicated.
Per-core device pipeline:
  1. Gather yts[j,o] = centroids[idxT[j,o]] via fused custom-DVE ops
     (2 codebook entries per instruction, 8 instructions per chain). Chains
     run on [128, 4096]-wide tiles (8 j-blocks packed along the free dim)
     to amortize per-op overhead; the DVE chain is the kernel's critical
     path for the first ~150us.
  2. W_u^T[i,o] = sum_j Pi[j,i] * yts[j,o] on the PE (bf16, f32 psum),
     j-major over 8 psum banks x 4 passes; pass 0 overlaps the gather.
  3. outT[o,t] = sum_i W_u^T[i,o] * xT[i,t] over 16 token groups of 512,
     4 psum chains per group alternating bank sets between groups (no
     bank-turnaround stall); out = rn*acc + bias split across DVE+GpSimd.
Host feeds x pre-transposed/bf16-cast (layout prep), Pi in column-stripe
layout, indices transposed+packed; host reassembles the 8 outT shards.
"""

import numpy as np

# Problem geometry (hardcoded per contract)
OUT, IN = 4096, 4096
B, S = 4, 2048
T = B * S          # 8192 tokens
NCORES = 8
P = 128            # partitions

_DVE_OPS = None
_NC_CACHE = {}


def _register_dve_ops():
    """Register the fused VQ-gather ops in dve_ops.OPS (idempotent).

    VQ_PAIR covers codebook entries {imm2, imm2+1}; VQ_ACC2 accumulates two
    more on top of Src1. 8 instructions cover all 16 entries. All bf16
    (exact: per element exactly one eq-term is nonzero, so every partial
    sum is 0 or bf16(c_k) — no rounding drift).
    """
    global _DVE_OPS
    if _DVE_OPS is not None:
        return _DVE_OPS
    import concourse.dve_ops as dvo
    from concourse.dve_spec import (
        Spec, Src0, Src1, C0, C1, C2, One, eq, lower,
    )
    from concourse.dve_uop import DveOpSpec

    existing = {op.name: op for op in dvo.OPS}
    if "VQ_PAIR" in existing:
        _DVE_OPS = {k: existing[k] for k in ("VQ_PAIR", "VQ_ACC2")}
        return _DVE_OPS

    ver = "v3"  # TRN2

    def mk(name, spec, rd1):
        opcode = dvo._CUSTOM_DVE_ROW_BASE + len(dvo.OPS)
        dvo._SUB_OPCODE_FOR_NAME[name] = opcode
        s = DveOpSpec(name=name, opcode=opcode, uops=lower(spec, ver=ver), rd1_en=rd1)
        op = dvo.DveOp(name, spec, subdim=False, uops_sha={ver: s.sha(ver)})
        dvo.OPS.append(op)
        dvo.CUSTOM_DVE_SPECS[name] = spec
        return op

    # out = (idx==imm2)*s0 + (idx==imm2+1)*s1
    pair = mk(
        "VQ_PAIR",
        Spec(
            body=eq(Src0, C2) * C0 + eq(Src0, C2 + One) * C1,
            reference=lambda in0, in1, s0, s1, imm2: (
                (in0 == imm2) * s0 + (in0 == imm2 + 1) * s1
            ).astype(np.float32),
        ),
        False,
    )
    # out = acc + (idx==imm2)*s0 + (idx==imm2+1)*s1
    acc = mk(
        "VQ_ACC2",
        Spec(
            body=Src1 + eq(Src0, C2) * C0 + eq(Src0, C2 + One) * C1,
            reference=lambda in0, in1, s0, s1, imm2: (
                in1 + (in0 == imm2) * s0 + (in0 == imm2 + 1) * s1
            ).astype(np.float32),
        ),
        True,
    )
    _DVE_OPS = {"VQ_PAIR": pair, "VQ_ACC2": acc}
    return _DVE_OPS


# Gather chain sizes (j-blocks per wide DVE chain). Ragged: the last chain
# is a single j-block so almost all of stage-2 pass 0 completes inside the
# gather window (only the final block's 8 matmuls trail the gather).
CHAIN_SIZES = (11, 11, 9, 1)


def build_nc(cvals, in_=IN, t=T, osh=OUT // NCORES, tch=512):
    """Build the SPMD Bass program. cvals: 16 python floats (codebook)."""
    import concourse.bacc as bacc
    import concourse.mybir as mybir
    from concourse.tile import TileContext

    f32 = mybir.dt.float32
    bf16 = mybir.dt.bfloat16

    nj = in_ // P          # j blocks (rows of Pi / x input dim)
    ni = in_ // P          # i blocks (cols of Pi / contraction of main mm)
    nob = osh // P         # output feature blocks per core (4)
    npass = 4              # stage-2 passes (8 psum banks each)
    nk = ni // npass       # i-blocks per stage-2 pass (8)
    ngt = t // tch         # stage-3 token groups (16)

    nc = bacc.Bacc()
    xT_d = nc.dram_tensor("xT", [in_, t], bf16, kind="ExternalInput")
    piR_d = nc.dram_tensor("PiR", [npass * nj, P, nk, P], bf16,
                           kind="ExternalInput")
    idxW_d = nc.dram_tensor("idxW", [P, nj * osh], bf16, kind="ExternalInput")
    rn_d = nc.dram_tensor("rn", [osh], f32, kind="ExternalInput")
    bias_d = nc.dram_tensor("bias", [osh], f32, kind="ExternalInput")
    outT_d = nc.dram_tensor("outT", [osh, t], f32, kind="ExternalOutput")

    with TileContext(nc) as tc:
        with (
            tc.tile_pool(name="constp", bufs=1) as constp,
            tc.tile_pool(name="idxp", bufs=2) as idxp,
            tc.tile_pool(name="ytsp", bufs=1) as ytsp,
            tc.tile_pool(name="pip", bufs=4) as pip,
            tc.tile_pool(name="wtp", bufs=1) as wtp,
            tc.tile_pool(name="xrp", bufs=14) as xrp,
            tc.tile_pool(name="outp", bufs=4) as outp,
            tc.tile_pool(name="wpsum", bufs=1, space="PSUM") as wpsum,
        ):
            # ---- Stage 1: codebook gather: yts[j][p, o] = centroids[idxT] --
            # 4 wide chains of 8 fused custom-DVE ops on [P, 4096] tiles
            # (8 j-blocks packed along the free dim). Issue the first idx
            # DMA before anything else: the DVE chain is the critical path.
            ops = _register_dve_ops()
            vq_pair, vq_acc2 = ops["VQ_PAIR"], ops["VQ_ACC2"]
            ytsw = []       # (j_block_offset, width_in_blocks, tile)
            off = 0
            for m, cs in enumerate(CHAIN_SIZES):
                wf = cs * osh
                idx_t = idxp.tile([P, wf], bf16, name="idx_t", tag="idx",
                                  bufs=2)
                nc.sync.dma_start(
                    idx_t[:], idxW_d[:, off * osh:(off + cs) * osh])
                cur = idxp.tile([P, wf], bf16, name="g", tag="g", bufs=2)
                nc.vector._custom_dve(
                    vq_pair, out=cur[:], in0=idx_t[:],
                    s0=float(cvals[0]), s1=float(cvals[1]), imm2=0.0,
                )
                for k in range(2, 16, 2):
                    if k == 14:
                        dst = ytsp.tile([P, wf], bf16, name="y_t",
                                        tag=f"yts{m}")
                    else:
                        dst = idxp.tile([P, wf], bf16, name="g", tag="g",
                                        bufs=2)
                    nc.vector._custom_dve(
                        vq_acc2, out=dst[:], in0=idx_t[:], in1=cur[:],
                        s0=float(cvals[k]), s1=float(cvals[k + 1]),
                        imm2=float(k),
                    )
                    cur = dst
                ytsw.append((off, cs, cur))
                off += cs
            assert off == nj

            rn_sb = constp.tile([P, nob], f32, name="rn_sb")
            nc.scalar.dma_start(rn_sb[:], rn_d.rearrange("(b p) -> p b", p=P))
            bias_sb = constp.tile([P, nob], f32, name="bias_sb")
            nc.scalar.dma_start(bias_sb[:],
                                bias_d.rearrange("(b p) -> p b", p=P))

            def yts_view(j):
                for o0, cs, tile in ytsw:
                    if o0 <= j < o0 + cs:
                        return tile[:, (j - o0) * osh:(j - o0 + 1) * osh]
                raise AssertionError(j)

            # ---- Stage 2: wt[i_blk][p_i, o] = sum_j Pi[j, i] * yts[j, o] ---
            # j-major over 8 psum banks: pass p covers i-blocks 8p..8p+7,
            # consuming yts as the gather produces them (pass 0 absorbs
            # PE work into the gather window). Pi arrives as per-(pass, j)
            # interleaved [P, 8, P] tiles (1MB resident vs 6MB stripes).
            wts = [None] * ni
            for p4 in range(npass):
                ps = [
                    wpsum.tile([P, osh], f32, name="wps", tag=f"wps{k}")
                    for k in range(nk)
                ]
                for j in range(nj):
                    pi_t = pip.tile([P, nk, P], bf16, name="pi_t", tag="pi")
                    nc.sync.dma_start(pi_t[:], piR_d[p4 * nj + j])
                    yv = yts_view(j)
                    for k in range(nk):
                        nc.tensor.matmul(
                            ps[k][:], pi_t[:, k, :], yv,
                            start=(j == 0), stop=(j == nj - 1),
                        )
                for k in range(nk):
                    i_blk = p4 * nk + k
                    wt_t = wtp.tile([P, osh], bf16, name="wt_t",
                                    tag=f"wt{i_blk}")
                    # Alternate the psum->sbuf drains between Act and DVE so
                    # pass p+1's bank reuse isn't gated on one serial queue.
                    if k % 2 == 0:
                        nc.scalar.copy(wt_t[:], ps[k][:])
                    else:
                        nc.vector.tensor_copy(wt_t[:], ps[k][:])
                    wts[i_blk] = wt_t

            # ---- Stage 3: outT[o, t] = rn[o] * sum_i wt[i,o]*xT[i,t] + b[o]
            # 16 groups of 512 tokens; per group 4 psum chains (one per
            # 128-row output block), alternating bank sets between
            # consecutive groups so group g+1's matmuls never wait on
            # group g's drain. Drain work (scale+bias) alternates between
            # DVE and GpSimd; xr DMAs alternate sync/scalar queues.
            for g in range(ngt):
                bank = (g % 2) * nob
                mps = [
                    wpsum.tile([P, tch], f32, name="mp",
                               tag=f"wps{bank + ob}")
                    for ob in range(nob)
                ]
                for i_blk in range(ni):
                    xr = xrp.tile([P, tch], bf16, name="xr", tag="xr")
                    eng = nc.sync if (i_blk % 2 == 0) else nc.gpsimd
                    eng.dma_start(
                        xr[:],
                        xT_d[i_blk * P:(i_blk + 1) * P,
                             g * tch:(g + 1) * tch],
                    )
                    for ob in range(nob):
                        nc.tensor.matmul(
                            mps[ob][:], wts[i_blk][:, ob * P:(ob + 1) * P],
                            xr[:],
                            start=(i_blk == 0), stop=(i_blk == ni - 1),
                        )
                for ob in range(nob):
                    o_t = outp.tile([P, tch], f32, name="o_t", tag="out")
                    if ob % 2 == 0:
                        nc.vector.tensor_scalar(
                            o_t[:], mps[ob][:], rn_sb[:, ob:ob + 1],
                            bias_sb[:, ob:ob + 1],
                            mybir.AluOpType.mult, mybir.AluOpType.add,
                        )
                    else:
                        # out = Identity(in*scale + bias) on the Act engine
                        nc.scalar.activation(
                            o_t[:], mps[ob][:],
                            mybir.ActivationFunctionType.Identity,
                            bias=bias_sb[:, ob:ob + 1],
                            scale=rn_sb[:, ob:ob + 1],
                        )
                    nc.scalar.dma_start(
                        outT_d[ob * P:(ob + 1) * P,
                               g * tch:(g + 1) * tch],
                        o_t[:],
                    )
    nc.compile()
    return nc


def _prep_inputs(x, indices, Pi, row_norms, bias):
    """Host-side layout prep + sharding. Returns list of per-core in_maps."""
    import ml_dtypes

    bf16 = ml_dtypes.bfloat16
    x2 = np.ascontiguousarray(
        np.asarray(x, np.float32).reshape(T, IN).T
    ).astype(bf16)  # (IN, T)
    nj = IN // P
    npass, nk = 4, (IN // P) // 4
    # piR[p*nj + j, jp, k, ii] = Pi[j*P + jp, (p*nk + k)*P + ii]
    piR = np.ascontiguousarray(
        np.asarray(Pi, np.float32).astype(bf16)
        .reshape(nj, P, npass, nk, P).transpose(2, 0, 1, 3, 4)
        .reshape(npass * nj, P, nk, P)
    )
    idxT = np.ascontiguousarray(np.asarray(indices).T).astype(bf16)  # (IN, OUT)
    rn = np.asarray(row_norms, np.float32)
    bs = np.asarray(bias, np.float32)

    osh = OUT // NCORES
    in_maps = []
    for c in range(NCORES):
        sl = slice(c * osh, (c + 1) * osh)
        # idxW[p, j_blk*osh + o] = idxT[j_blk*P + p, c*osh + o]
        idxW = np.ascontiguousarray(
            idxT[:, sl].reshape(nj, P, osh).transpose(1, 0, 2)
            .reshape(P, nj * osh)
        )
        in_maps.append({
            "xT": x2,
            "PiR": piR,
            "idxW": idxW,
            "rn": np.ascontiguousarray(rn[sl]),
            "bias": np.ascontiguousarray(bs[sl]),
        })
    return in_maps


def _get_nc(centroids):
    key = np.asarray(centroids, np.float32).tobytes()
    nc = _NC_CACHE.get(key)
    if nc is None:
        cvals = [float(v) for v in np.asarray(centroids, np.float32)]
        assert len(cvals) == 16
        nc = build_nc(cvals)
        _NC_CACHE.clear()
        _NC_CACHE[key] = nc
    return nc


def kernel(x, indices, centroids, Pi, row_norms, bias):
    from concourse.bass_utils import run_bass_kernel_spmd

    nc = _get_nc(centroids)
    in_maps = _prep_inputs(x, indices, Pi, row_norms, bias)
    res = run_bass_kernel_spmd(nc, in_maps, list(range(NCORES)))
    shards = [np.asarray(res.results[c]["outT"]) for c in range(NCORES)]
    full = np.concatenate(shards, axis=0)           # (OUT, T)
    out = np.ascontiguousarray(full.T).reshape(B, S, OUT)
    return out.astype(np.float32)


# revision 13
# speedup vs baseline: 1.0728x; 1.0069x over previous
"""Trainium2 Bass kernel for CompressedLinear (VQ codebook linear layer).

Computes: out = x @ W^T + bias, where
  W = (centroids[indices] @ Pi) * row_norms[:, None]

Sharding: out_features (4096) split across 8 cores (512 each); x replicated.
Per-core device pipeline:
  1. Gather yts[j,o] = centroids[idxT[j,o]] via fused custom-DVE ops
     (2 codebook entries per instruction, 8 instructions per chain). Chains
     run on [128, 4096]-wide tiles (8 j-blocks packed along the free dim)
     to amortize per-op overhead; the DVE chain is the kernel's critical
     path for the first ~150us.
  2. W_u^T[i,o] = sum_j Pi[j,i] * yts[j,o] on the PE (bf16, f32 psum),
     j-major over 8 psum banks x 4 passes; pass 0 overlaps the gather.
  3. outT[o,t] = sum_i W_u^T[i,o] * xT[i,t] over 16 token groups of 512,
     4 psum chains per group alternating bank sets between groups (no
     bank-turnaround stall); out = rn*acc + bias split across DVE+GpSimd.
Host feeds x pre-transposed/bf16-cast (layout prep), Pi in column-stripe
layout, indices transposed+packed; host reassembles the 8 outT shards.
"""

import numpy as np

# Problem geometry (hardcoded per contract)
OUT, IN = 4096, 4096
B, S = 4, 2048
T = B * S          # 8192 tokens
NCORES = 8
P = 128            # partitions

_DVE_OPS = None
_NC_CACHE = {}


def _register_dve_ops():
    """Register the fused VQ-gather ops in dve_ops.OPS (idempotent).

    VQ_PAIR covers codebook entries {imm2, imm2+1}; VQ_ACC2 accumulates two
    more on top of Src1. 8 instructions cover all 16 entries. All bf16
    (exact: per element exactly one eq-term is nonzero, so every partial
    sum is 0 or bf16(c_k) — no rounding drift).
    """
    global _DVE_OPS
    if _DVE_OPS is not None:
        return _DVE_OPS
    import concourse.dve_ops as dvo
    from concourse.dve_spec import (
        Spec, Src0, Src1, C0, C1, C2, One, eq, lower,
    )
    from concourse.dve_uop import DveOpSpec

    existing = {op.name: op for op in dvo.OPS}
    if "VQ_PAIR" in existing:
        _DVE_OPS = {k: existing[k] for k in ("VQ_PAIR", "VQ_ACC2")}
        return _DVE_OPS

    ver = "v3"  # TRN2

    def mk(name, spec, rd1):
        opcode = dvo._CUSTOM_DVE_ROW_BASE + len(dvo.OPS)
        dvo._SUB_OPCODE_FOR_NAME[name] = opcode
        s = DveOpSpec(name=name, opcode=opcode, uops=lower(spec, ver=ver), rd1_en=rd1)
        op = dvo.DveOp(name, spec, subdim=False, uops_sha={ver: s.sha(ver)})
        dvo.OPS.append(op)
        dvo.CUSTOM_DVE_SPECS[name] = spec
        return op

    # out = (idx==imm2)*s0 + (idx==imm2+1)*s1
    pair = mk(
        "VQ_PAIR",
        Spec(
            body=eq(Src0, C2) * C0 + eq(Src0, C2 + One) * C1,
            reference=lambda in0, in1, s0, s1, imm2: (
                (in0 == imm2) * s0 + (in0 == imm2 + 1) * s1
            ).astype(np.float32),
        ),
        False,
    )
    # out = acc + (idx==imm2)*s0 + (idx==imm2+1)*s1
    acc = mk(
        "VQ_ACC2",
        Spec(
            body=Src1 + eq(Src0, C2) * C0 + eq(Src0, C2 + One) * C1,
            reference=lambda in0, in1, s0, s1, imm2: (
                in1 + (in0 == imm2) * s0 + (in0 == imm2 + 1) * s1
            ).astype(np.float32),
        ),
        True,
    )
    _DVE_OPS = {"VQ_PAIR": pair, "VQ_ACC2": acc}
    return _DVE_OPS


# Gather chain sizes (j-blocks per wide DVE chain). Ragged: a tiny first
# chain starts the DVE ~7us earlier (its idx DMA is 256KB, not 1.4MB); a
# single-block last chain means only the final j-block's 8 stage-2 pass-0
# matmuls trail the gather window.
CHAIN_SIZES = (2, 11, 11, 7, 1)


def build_nc(cvals, in_=IN, t=T, osh=OUT // NCORES, tch=512):
    """Build the SPMD Bass program. cvals: 16 python floats (codebook)."""
    import concourse.bacc as bacc
    import concourse.mybir as mybir
    from concourse.tile import TileContext

    f32 = mybir.dt.float32
    bf16 = mybir.dt.bfloat16

    nj = in_ // P          # j blocks (rows of Pi / x input dim)
    ni = in_ // P          # i blocks (cols of Pi / contraction of main mm)
    nob = osh // P         # output feature blocks per core (4)
    npass = 4              # stage-2 passes (8 psum banks each)
    nk = ni // npass       # i-blocks per stage-2 pass (8)
    ngt = t // tch         # stage-3 token groups (16)

    nc = bacc.Bacc()
    xT_d = nc.dram_tensor("xT", [in_, t], bf16, kind="ExternalInput")
    piR_d = nc.dram_tensor("PiR", [npass * nj, P, nk, P], bf16,
                           kind="ExternalInput")
    idxW_d = nc.dram_tensor("idxW", [P, nj * osh], bf16, kind="ExternalInput")
    rn_d = nc.dram_tensor("rn", [osh], f32, kind="ExternalInput")
    bias_d = nc.dram_tensor("bias", [osh], f32, kind="ExternalInput")
    outT_d = nc.dram_tensor("outT", [osh, t], f32, kind="ExternalOutput")

    with TileContext(nc) as tc:
        with (
            tc.tile_pool(name="constp", bufs=1) as constp,
            tc.tile_pool(name="idxp", bufs=2) as idxp,
            tc.tile_pool(name="ytsp", bufs=1) as ytsp,
            tc.tile_pool(name="pip", bufs=6) as pip,
            tc.tile_pool(name="wtp", bufs=1) as wtp,
            tc.tile_pool(name="xrp", bufs=20) as xrp,
            tc.tile_pool(name="outp", bufs=4) as outp,
            tc.tile_pool(name="wpsum", bufs=1, space="PSUM") as wpsum,
        ):
            # ---- Stage 1: codebook gather: yts[j][p, o] = centroids[idxT] --
            # 4 wide chains of 8 fused custom-DVE ops on [P, 4096] tiles
            # (8 j-blocks packed along the free dim). Issue the first idx
            # DMA before anything else: the DVE chain is the critical path.
            ops = _register_dve_ops()
            vq_pair, vq_acc2 = ops["VQ_PAIR"], ops["VQ_ACC2"]
            ytsw = []       # (j_block_offset, width_in_blocks, tile)
            off = 0
            for m, cs in enumerate(CHAIN_SIZES):
                wf = cs * osh
                idx_t = idxp.tile([P, wf], bf16, name="idx_t", tag="idx",
                                  bufs=2)
                nc.sync.dma_start(
                    idx_t[:], idxW_d[:, off * osh:(off + cs) * osh])
                cur = idxp.tile([P, wf], bf16, name="g", tag="g", bufs=2)
                nc.vector._custom_dve(
                    vq_pair, out=cur[:], in0=idx_t[:],
                    s0=float(cvals[0]), s1=float(cvals[1]), imm2=0.0,
                )
                for k in range(2, 16, 2):
                    if k == 14:
                        dst = ytsp.tile([P, wf], bf16, name="y_t",
                                        tag=f"yts{m}")
                    else:
                        dst = idxp.tile([P, wf], bf16, name="g", tag="g",
                                        bufs=2)
                    nc.vector._custom_dve(
                        vq_acc2, out=dst[:], in0=idx_t[:], in1=cur[:],
                        s0=float(cvals[k]), s1=float(cvals[k + 1]),
                        imm2=float(k),
                    )
                    cur = dst
                ytsw.append((off, cs, cur))
                off += cs
            assert off == nj

            rn_sb = constp.tile([P, nob], f32, name="rn_sb")
            nc.scalar.dma_start(rn_sb[:], rn_d.rearrange("(b p) -> p b", p=P))
            bias_sb = constp.tile([P, nob], f32, name="bias_sb")
            nc.scalar.dma_start(bias_sb[:],
                                bias_d.rearrange("(b p) -> p b", p=P))

            def yts_view(j):
                for o0, cs, tile in ytsw:
                    if o0 <= j < o0 + cs:
                        return tile[:, (j - o0) * osh:(j - o0 + 1) * osh]
                raise AssertionError(j)

            # ---- Stage 2: wt[i_blk][p_i, o] = sum_j Pi[j, i] * yts[j, o] ---
            # j-major over 8 psum banks: pass p covers i-blocks 8p..8p+7,
            # consuming yts as the gather produces them (pass 0 absorbs
            # PE work into the gather window). Pi arrives as per-(pass, j)
            # interleaved [P, 8, P] tiles (1MB resident vs 6MB stripes).
            wts = [None] * ni
            for p4 in range(npass):
                ps = [
                    wpsum.tile([P, osh], f32, name="wps", tag=f"wps{k}")
                    for k in range(nk)
                ]
                for j in range(nj):
                    pi_t = pip.tile([P, nk, P], bf16, name="pi_t", tag="pi")
                    nc.sync.dma_start(pi_t[:], piR_d[p4 * nj + j])
                    yv = yts_view(j)
                    for k in range(nk):
                        nc.tensor.matmul(
                            ps[k][:], pi_t[:, k, :], yv,
                            start=(j == 0), stop=(j == nj - 1),
                        )
                for k in range(nk):
                    i_blk = p4 * nk + k
                    wt_t = wtp.tile([P, osh], bf16, name="wt_t",
                                    tag=f"wt{i_blk}")
                    # Alternate the psum->sbuf drains between Act and DVE so
                    # pass p+1's bank reuse isn't gated on one serial queue.
                    if k % 2 == 0:
                        nc.scalar.copy(wt_t[:], ps[k][:])
                    else:
                        nc.vector.tensor_copy(wt_t[:], ps[k][:])
                    wts[i_blk] = wt_t

            # ---- Stage 3: outT[o, t] = rn[o] * sum_i wt[i,o]*xT[i,t] + b[o]
            # 16 groups of 512 tokens; per group 4 psum chains (one per
            # 128-row output block), alternating bank sets between
            # consecutive groups so group g+1's matmuls never wait on
            # group g's drain. Drain work (scale+bias) alternates between
            # DVE and GpSimd; xr DMAs alternate sync/scalar queues.
            for g in range(ngt):
                bank = (g % 2) * nob
                mps = [
                    wpsum.tile([P, tch], f32, name="mp",
                               tag=f"wps{bank + ob}")
                    for ob in range(nob)
                ]
                for i_blk in range(ni):
                    xr = xrp.tile([P, tch], bf16, name="xr", tag="xr")
                    eng = nc.sync if (i_blk % 2 == 0) else nc.gpsimd
                    eng.dma_start(
                        xr[:],
                        xT_d[i_blk * P:(i_blk + 1) * P,
                             g * tch:(g + 1) * tch],
                    )
                    for ob in range(nob):
                        nc.tensor.matmul(
                            mps[ob][:], wts[i_blk][:, ob * P:(ob + 1) * P],
                            xr[:],
                            start=(i_blk == 0), stop=(i_blk == ni - 1),
                        )
                for ob in range(nob):
                    o_t = outp.tile([P, tch], f32, name="o_t", tag="out")
                    if ob % 2 == 0:
                        nc.vector.tensor_scalar(
                            o_t[:], mps[ob][:], rn_sb[:, ob:ob + 1],
                            bias_sb[:, ob:ob + 1],
                            mybir.AluOpType.mult, mybir.AluOpType.add,
                        )
                    else:
                        # out = Identity(in*scale + bias) on the Act engine
                        nc.scalar.activation(
                            o_t[:], mps[ob][:],
                            mybir.ActivationFunctionType.Identity,
                            bias=bias_sb[:, ob:ob + 1],
                            scale=rn_sb[:, ob:ob + 1],
                        )
                    nc.scalar.dma_start(
                        outT_d[ob * P:(ob + 1) * P,
                               g * tch:(g + 1) * tch],
                        o_t[:],
                    )
    nc.compile()
    return nc


def _prep_inputs(x, indices, Pi, row_norms, bias):
    """Host-side layout prep + sharding. Returns list of per-core in_maps."""
    import ml_dtypes

    bf16 = ml_dtypes.bfloat16
    x2 = np.ascontiguousarray(
        np.asarray(x, np.float32).reshape(T, IN).T
    ).astype(bf16)  # (IN, T)
    nj = IN // P
    npass, nk = 4, (IN // P) // 4
    # piR[p*nj + j, jp, k, ii] = Pi[j*P + jp, (p*nk + k)*P + ii]
    piR = np.ascontiguousarray(
        np.asarray(Pi, np.float32).astype(bf16)
        .reshape(nj, P, npass, nk, P).transpose(2, 0, 1, 3, 4)
        .reshape(npass * nj, P, nk, P)
    )
    idxT = np.ascontiguousarray(np.asarray(indices).T).astype(bf16)  # (IN, OUT)
    rn = np.asarray(row_norms, np.float32)
    bs = np.asarray(bias, np.float32)

    osh = OUT // NCORES
    in_maps = []
    for c in range(NCORES):
        sl = slice(c * osh, (c + 1) * osh)
        # idxW[p, j_blk*osh + o] = idxT[j_blk*P + p, c*osh + o]
        idxW = np.ascontiguousarray(
            idxT[:, sl].reshape(nj, P, osh).transpose(1, 0, 2)
            .reshape(P, nj * osh)
        )
        in_maps.append({
            "xT": x2,
            "PiR": piR,
            "idxW": idxW,
            "rn": np.ascontiguousarray(rn[sl]),
            "bias": np.ascontiguousarray(bs[sl]),
        })
    return in_maps


def _get_nc(centroids):
    key = np.asarray(centroids, np.float32).tobytes()
    nc = _NC_CACHE.get(key)
    if nc is None:
        cvals = [float(v) for v in np.asarray(centroids, np.float32)]
        assert len(cvals) == 16
        nc = build_nc(cvals)
        _NC_CACHE.clear()
        _NC_CACHE[key] = nc
    return nc


def kernel(x, indices, centroids, Pi, row_norms, bias):
    from concourse.bass_utils import run_bass_kernel_spmd

    nc = _get_nc(centroids)
    in_maps = _prep_inputs(x, indices, Pi, row_norms, bias)
    res = run_bass_kernel_spmd(nc, in_maps, list(range(NCORES)))
    shards = [np.asarray(res.results[c]["outT"]) for c in range(NCORES)]
    full = np.concatenate(shards, axis=0)           # (OUT, T)
    out = np.ascontiguousarray(full.T).reshape(B, S, OUT)
    return out.astype(np.float32)
